# revision 13
# baseline (speedup 1.0000x reference)
"""Trainium2 Bass kernel for nn_MultiHeadAttention_2259152798076.

Faithful to the reference (source bug included): Q = K = V = x @ W_k.T;
W_q / W_v are unused.

Sharding: data-parallel over batch B=8 -> one batch per NeuronCore, tiny
128x128 weights replicated. Inside each core:

  h = x_b @ W_k.T                      [2048, 128]  (8 heads x d_k=16)
  per head: P = exp(h_h h_h^T / 4)     (no max-subtraction; scores ~ N(0,1))
  ctx_h = P h_h / rowsum(P)
  y_b = concat_h(ctx_h) @ W_o.T

Kernel strategy (per core):
  * "spread" layout: hT with 4 heads per tensor placed at 32-partition
    boundaries (16 data rows + 16 zero rows per group) so scores for 4
    heads run as concurrent row-tiled (tile_position=(32g,0)) matmuls.
  * scores are computed directly in TRANSPOSED tile layout [k, q] using
    the symmetry of h h^T, so the ctx matmul needs no transposes.
  * exp on ScalarE reads the scores PSUM tile [128, 1024] (4 heads) in
    one instruction and writes PT to SBUF.
  * ctx^T accumulated over k-chunks via col-tiled (tile_position=(0,32g))
    matmuls with an extra all-ones column in lhsT producing the softmax
    denominators l[q] as a 17th output row for free.
  * normalization: l rows gathered by an indicator matmul, reciprocal on
    DVE, broadcast back over partitions by a second indicator matmul,
    then one elementwise multiply per ctx tile.
  * output projection: 8 accumulating K=16 matmuls per 128-query tile.
"""

import numpy as np

B, S, D, H, DK = 8, 2048, 128, 8, 16
NCH = S // 128          # 16 k-chunks of 128
NJJ = S // 256          # 8 q-pairs of 256
_CACHE = {}


def _build(dt_fast_name="bfloat16"):
    import concourse.bacc as bacc
    import concourse.mybir as mybir
    from concourse import masks
    from concourse.alu_op_type import AluOpType
    from concourse.tile import TileContext

    F32 = mybir.dt.float32
    DTF = getattr(mybir.dt, dt_fast_name)
    EXP = mybir.ActivationFunctionType.Exp

    nc = bacc.Bacc("TRN2", target_bir_lowering=False, debug=False, num_devices=8)

    x = nc.dram_tensor("x", [S, D], F32, kind="ExternalInput")
    wk = nc.dram_tensor("wk", [D, D], F32, kind="ExternalInput")
    wo = nc.dram_tensor("wo", [D, D], F32, kind="ExternalInput")
    indg = nc.dram_tensor("indg", [128, 8], F32, kind="ExternalInput")
    indb = nc.dram_tensor("indb", [8, 128], F32, kind="ExternalInput")
    y = nc.dram_tensor("y", [S, D], F32, kind="ExternalOutput")

    with TileContext(nc) as tc:
        with (
            tc.tile_pool(name="persist", bufs=1) as sb,
            tc.tile_pool(name="work", bufs=2) as wk_pool,
        ):
            ident = sb.tile([128, 128], F32)
            masks.make_identity(nc, ident[:])

            x_sb = wk_pool.tile([128, NCH * 128], F32, tag="xin", bufs=1)
            nc.sync.dma_start(
                out=x_sb[:].rearrange("p (n m) -> p n m", m=128),
                in_=x.rearrange("(n p) m -> p n m", p=128),
            )
            wk_sb = sb.tile([128, 128], F32)
            wo_sb = sb.tile([128, 128], F32)
            indg_sb = sb.tile([128, 8], F32)
            indb_sb = [sb.tile([4, 128], F32, name=f"indb{h}") for h in range(2)]
            nc.sync.dma_start(out=wk_sb[:], in_=wk[:])
            nc.sync.dma_start(out=wo_sb[:], in_=wo[:])
            nc.sync.dma_start(out=indg_sb[:], in_=indg[:])
            for h in range(2):
                nc.sync.dma_start(out=indb_sb[h][:], in_=indb[4 * h : 4 * (h + 1), :])

            wkT = sb.tile([128, 128], F32)
            woT = sb.tile([128, 128], F32)
            # spread weight layouts: wkTs[half][:, 32g:32g+16] = wkT cols of
            # head 4*half+g; other columns zero -> matmul output lands in
            # spread partition layout directly.
            wkTs = [sb.tile([128, 128], F32, name=f"wkTs{h}") for h in range(2)]
            # woTs[half]: rows 32g..32g+16 = W_o.T rows of head 4*half+g,
            # other rows zero -> K=128 out-proj matmul vs full spread ctx tile
            wos = [sb.tile([128, 128], F32, name=f"wos{h}") for h in range(2)]
            woTs = [sb.tile([128, 128], F32, name=f"woTs{h}") for h in range(2)]
            xT = sb.tile([128, S], F32)
            ones8 = sb.tile([128, 8], F32)
            spread = [sb.tile([128, S], DTF, name=f"spread{h}") for h in range(2)]
            haug = sb.tile([128, NCH * 136], DTF)

            with tc.tile_pool(name="initps", bufs=2, space="PSUM") as ips:
                nc.vector.memset(ones8[:], 1.0)
                tp = ips.tile([128, 128], F32, tag="t")
                nc.tensor.transpose(tp[:], wk_sb[:], ident[:])
                nc.vector.tensor_copy(wkT[:], tp[:])
                tp2 = ips.tile([128, 128], F32, tag="t")
                nc.tensor.transpose(tp2[:], wo_sb[:], ident[:])
                nc.vector.tensor_copy(woT[:], tp2[:])

                for h in range(2):
                    nc.vector.memset(wkTs[h][:], 0.0)
                    nc.vector.tensor_copy(
                        wkTs[h][:].rearrange("p (g c) -> p g c", c=32)[:, :, 0:16],
                        wkT[:, 64 * h : 64 * (h + 1)].rearrange(
                            "p (g c) -> p g c", c=16
                        ),
                    )
                    # spread W_o columns, then transpose -> row-spread W_o.T
                    nc.vector.memset(wos[h][:], 0.0)
                    nc.vector.tensor_copy(
                        wos[h][:].rearrange("p (g c) -> p g c", c=32)[:, :, 0:16],
                        wo_sb[:, 64 * h : 64 * (h + 1)].rearrange(
                            "p (g c) -> p g c", c=16
                        ),
                    )
                    tph = ips.tile([128, 128], F32, tag="t")
                    nc.tensor.transpose(tph[:], wos[h][:], ident[:])
                    nc.vector.tensor_copy(woTs[h][:], tph[:])

                # xT via 16 PE transposes
                for n in range(NCH):
                    tpn = ips.tile([128, 128], F32, tag="t")
                    nc.tensor.transpose(
                        tpn[:], x_sb[:, 128 * n : 128 * (n + 1)], ident[:]
                    )
                    nc.vector.tensor_copy(xT[:, 128 * n : 128 * (n + 1)], tpn[:])

                # spread[half] = (wkTs[half].T @ xT) rounded to DTF
                for h in range(2):
                    sp = ips.tile([128, S], F32, tag="sp", bufs=1)
                    for j in range(4):
                        nc.tensor.matmul(
                            sp[:, 512 * j : 512 * (j + 1)],
                            wkTs[h][:],
                            xT[:, 512 * j : 512 * (j + 1)],
                            start=True,
                            stop=True,
                        )
                    nc.vector.tensor_copy(spread[h][:], sp[:])

                # h natural chunks + ones column -> haug (chunk n at 136*n,
                # head hh slice of 17 = 16 dims + one)
                for n in range(NCH):
                    hp = ips.tile([128, 128], F32, tag="t")
                    nc.tensor.matmul(
                        hp[:],
                        xT[:, 128 * n : 128 * (n + 1)],
                        wkT[:],
                        start=True,
                        stop=True,
                    )
                    blk = haug[:, 136 * n : 136 * (n + 1)].rearrange(
                        "p (hh c) -> p hh c", c=17
                    )
                    nc.vector.tensor_copy(
                        blk[:, :, 0:16],
                        hp[:].rearrange("p (hh c) -> p hh c", c=16),
                    )
                    nc.vector.tensor_copy(
                        blk[:, :, 16:17],
                        ones8[:].rearrange("p (a b) -> p a b", b=1),
                    )

            with (
                tc.tile_pool(name="sps", bufs=2, space="PSUM") as sps,
                tc.tile_pool(name="ctxps", bufs=1, space="PSUM") as cps,
                tc.tile_pool(name="miscps", bufs=2, space="PSUM") as mps,
                tc.tile_pool(name="ptpool", bufs=3) as ptp,
                tc.tile_pool(name="tailsb", bufs=2) as tsb,
            ):
                QW = 512
                for jj in range(S // QW):
                    q0 = QW * jj
                    ctx_ps = [
                        cps.tile([128, QW], F32, name=f"ctx{jj}_{h}", tag=f"ctx{h}")
                        for h in range(2)
                    ]
                    for i in range(NCH):
                        k0 = 128 * i
                        # 4 passes of 2 heads; each row-group matmul fills a
                        # full exclusive PSUM bank [128, 512]
                        for p in range(4):
                            h, g0 = p // 2, 2 * (p % 2)
                            s_ps = sps.tile([128, 1024], F32, tag="s")
                            for dg in range(2):
                                g = g0 + dg
                                nc.tensor.matmul(
                                    s_ps[:, 512 * dg : 512 * (dg + 1)],
                                    spread[h][32 * g : 32 * (g + 1), k0 : k0 + 128],
                                    spread[h][32 * g : 32 * (g + 1), q0 : q0 + QW],
                                    start=True,
                                    stop=True,
                                    tile_position=(32 * g, 0),
                                )
                            pt = ptp.tile([128, 1024], DTF, tag="pt")
                            nc.scalar.activation(pt[:], s_ps[:], EXP, scale=0.25)
                            for dg in range(2):
                                g = g0 + dg
                                hh = 4 * h + g
                                nc.tensor.matmul(
                                    ctx_ps[h][32 * g : 32 * g + 17, :],
                                    haug[:, 136 * i : 136 * (i + 1)].rearrange(
                                        "p (w c) -> p w c", c=17
                                    )[:, hh, :],
                                    pt[:, 512 * dg : 512 * (dg + 1)],
                                    start=(i == 0),
                                    stop=(i == NCH - 1),
                                    tile_position=(0, 32 * g),
                                    skip_group_check=True,
                                )

                    # --- normalization + output projection for this q-group
                    ctx_sb = [
                        tsb.tile([128, QW], F32, name=f"ctxsb{jj}_{h}", tag=f"cs{h}")
                        for h in range(2)
                    ]
                    for h in range(2):
                        nc.vector.memset(ctx_sb[h][:], 0.0)
                        for g in range(4):
                            nc.vector.tensor_copy(
                                ctx_sb[h][32 * g : 32 * g + 17, :],
                                ctx_ps[h][32 * g : 32 * g + 17, :],
                            )
                    for h in range(2):
                        l4 = mps.tile([4, QW], F32, tag="m")
                        nc.tensor.matmul(
                            l4[:],
                            indg_sb[:, 4 * h : 4 * (h + 1)],
                            ctx_sb[h][:],
                            start=True,
                            stop=True,
                        )
                        r4 = tsb.tile([4, QW], F32, tag="r4")
                        nc.vector.reciprocal(r4[:], l4[:])
                        rb = mps.tile([128, QW], F32, tag="m")
                        nc.tensor.matmul(
                            rb[:], indb_sb[h][:], r4[:], start=True, stop=True
                        )
                        nc.vector.tensor_tensor(
                            ctx_sb[h][:], ctx_sb[h][:], rb[:], AluOpType.mult
                        )

                    for qt in range(QW // 128):
                        op = mps.tile([128, 128], F32, tag="m")
                        for h in range(2):
                            nc.tensor.matmul(
                                op[:],
                                ctx_sb[h][:, 128 * qt : 128 * (qt + 1)],
                                woTs[h][:],
                                start=(h == 0),
                                stop=(h == 1),
                            )
                        o_sb = tsb.tile([128, 128], F32, tag="osb")
                        nc.vector.tensor_copy(o_sb[:], op[:])
                        nc.sync.dma_start(
                            out=y[q0 + 128 * qt : q0 + 128 * (qt + 1), :],
                            in_=o_sb[:],
                        )

    nc.compile()
    return nc


def _host_consts():
    indg = np.zeros((128, 8), np.float32)
    for h in range(2):
        for g in range(4):
            indg[32 * g + 16, 4 * h + g] = 1.0
    indb = np.zeros((8, 128), np.float32)
    for h in range(2):
        for g in range(4):
            indb[4 * h + g, 32 * g : 32 * g + 17] = 1.0
    return indg, indb


def kernel(x, W_k, W_q, W_v, W_o):
    from concourse.bass_utils import run_bass_kernel_spmd

    if "nc" not in _CACHE:
        _CACHE["nc"] = _build()
    nc = _CACHE["nc"]

    indg, indb = _host_consts()
    wk = np.ascontiguousarray(np.asarray(W_k, dtype=np.float32))
    wo = np.ascontiguousarray(np.asarray(W_o, dtype=np.float32))
    xs = np.ascontiguousarray(np.asarray(x, dtype=np.float32))
    in_maps = [
        {"x": xs[b], "wk": wk, "wo": wo, "indg": indg, "indb": indb}
        for b in range(B)
    ]
    res = run_bass_kernel_spmd(nc, in_maps, core_ids=list(range(B)))
    return np.stack([res.results[b]["y"] for b in range(B)], axis=0)
